# revision 16
# baseline (speedup 1.0000x reference)
"""Bass/Trainium2 kernel for nn_BasicBlock_73933567033945 (CDConv / gnn_message_passing).

Strategy: graph is a fixed +-8 sequence window inside each of 4 chains of
L=2048 nodes (verified against the src/dst inputs at runtime). Shard the
8192 nodes across 8 NeuronCores (1024 nodes each) with an 8-node halo.

Per core:
  Phase A: hT[w, m] = lrelu(W_in^T @ lrelu(x)^T) computed directly in
    transposed form from a host-transposed x slab (no PE transposes).
  Phase A2: G[m, o*24+c] = sum_w hT[w, m] * Wk[c*32+w, o] (PE matmuls,
    hT slices stationary). G folds the output projection Wk into the
    gathered features, so the per-edge bilinear becomes
      conv[n, o] = sum_{k,c} kern[n, k, c] * G[n+k, o*24+c]
    (G slab row m of tile t holds node 112t + m - 8).
  Phase B per 112-node tile, products anchored at the G partition m:
    geometry is computed *center-shifted* -- the center pos|ori are
    gathered with down-shift matmuls so partition m holds delta of edge
    (n=m-k, k); the block-diagonal WS matmul then yields
    kern2[m, (k,c)] = kern(m-k, k, c) with no extra shifting. The DVE
    runs 17 all-bf16 contiguous multiplies tm_k[m] = G[m] * kern2[m,k],
    and 17 shift matmuls accumulate tm_k into PSUM re-aligned to the
    output node (out[n] += tm_k[n+k]), fusing the k-sum into the PE.
    A single 24-wide grouped reduce finishes the c-contraction.
    No per-k transposes, no Wk matmuls, no DVE add chains.
"""
import numpy as np

B, L, C = 4, 2048, 128
N = B * L
W = 32
KC = 24
SEQ_L = 11
R = 12.0
WIN = 8
NEG_IN = 0.1
NEG_K = 0.2
NCORES = 8
NPC = N // NCORES          # 1024 nodes per core
TS = 112                   # output nodes per tile
NT = 10                    # tiles per core (9*112 + 16)
HALO = 1152                # padded halo rows per core (needs 1136)
K17 = 2 * WIN + 1          # 17 window offsets
S_HALF = SEQ_L // 2
GW = W * KC                # 768 = width of a G slab / tm row

_PROG = {}


def _sidx(k):
    return int(np.clip(k - WIN, -S_HALF, S_HALF)) + S_HALF


def _build_program():
    import concourse.tile as tile
    from concourse import mybir, bacc
    from concourse.bass_utils import run_bass_kernel_spmd  # noqa: F401
    from contextlib import ExitStack

    f32 = mybir.dt.float32
    bf16 = mybir.dt.bfloat16
    AF = mybir.ActivationFunctionType
    OP = mybir.AluOpType
    AX = mybir.AxisListType

    nc = bacc.Bacc("TRN2", target_bir_lowering=False, debug=False)

    def din(name, shape):
        return nc.dram_tensor(name, shape, f32, kind="ExternalInput").ap()

    xT_slot = din("xT_slot", [128, HALO])
    xc_slot = din("xc_slot", [128, NT * C])
    po_slot = din("po_slot", [128, NT * 12])
    w_in = din("w_in", [C, W])
    ws_a = din("ws_a", [128, K17 * KC])
    ws_b = din("ws_b", [8, K17 * KC])
    wkoc = din("wkoc", [W, GW])
    w_out = din("w_out", [W, C])
    ident = din("ident", [128, 128])
    shifts_c = din("shifts_c", [128, K17 * 128])
    shifts_s = din("shifts_s", [128, K17 * TS])
    w5r = din("w5r", [128, 3 * KC])
    b5r = din("b5r", [128, KC])
    maskd = din("maskd", [128, NT * K17])
    nclmp = din("nclmp", [128, NT])
    alph1 = din("alph1", [128, 1])
    alph2 = din("alph2", [128, 1])
    y = nc.dram_tensor("y", [NPC, C], f32, kind="ExternalOutput").ap()

    with tile.TileContext(nc) as tc, ExitStack() as ctx:
        pers = ctx.enter_context(tc.tile_pool(name="pers", bufs=1))

        def load(ap_in, shape, tag):
            t = pers.tile(shape, f32, tag=tag)
            nc.sync.dma_start(t[:], ap_in)
            return t

        xT_sb = load(xT_slot, [128, HALO], "xT")
        xc_all = load(xc_slot, [128, NT * C], "xc_all")
        po_all = load(po_slot, [128, NT * 12], "po_all")
        w_in_f = load(w_in, [C, W], "w_in")
        ws_a_f = load(ws_a, [128, K17 * KC], "ws_a")
        ws_b_f = load(ws_b, [8, K17 * KC], "ws_b")
        wkoc_f = load(wkoc, [W, GW], "wkoc")
        w_out_f = load(w_out, [W, C], "w_out")
        id_sb = load(ident, [128, 128], "ident")
        shc_sb = load(shifts_c, [128, K17 * 128], "shifts_c")
        shs_f = load(shifts_s, [128, K17 * TS], "shifts_s")
        w5r_sb = load(w5r, [128, 3 * KC], "w5r")
        b5r_sb = load(b5r, [128, KC], "b5r")
        mask_sb = load(maskd, [128, NT * K17], "mask")
        ncl_sb = load(nclmp, [128, NT], "nclmp")
        a1_sb = load(alph1, [128, 1], "a1")
        a2_sb = load(alph2, [128, 1], "a2")

        # bf16 casts of shared weights
        w_in_b = pers.tile([C, W], bf16, tag="w_in_b")
        nc.vector.tensor_copy(w_in_b[:], w_in_f[:])
        ws_a_b = pers.tile([128, K17 * KC], bf16, tag="ws_a_b")
        nc.vector.tensor_copy(ws_a_b[:], ws_a_f[:])
        ws_b_b = pers.tile([8, K17 * KC], bf16, tag="ws_b_b")
        nc.vector.tensor_copy(ws_b_b[:], ws_b_f[:])
        wkoc_b = pers.tile([W, GW], bf16, tag="wkoc_b")
        nc.vector.tensor_copy(wkoc_b[:], wkoc_f[:])
        w_out_b = pers.tile([W, C], bf16, tag="w_out_b")
        nc.vector.tensor_copy(w_out_b[:], w_out_f[:])
        shs_b = pers.tile([128, K17 * TS], bf16, tag="shs_b")
        nc.vector.tensor_copy(shs_b[:], shs_f[:])

        hT = pers.tile([W, HALO], bf16, tag="hT")
        G = pers.tile([128, NT * GW], bf16, tag="G")

        # ---------------- Phase A: hT = lrelu(W_in^T @ lrelu(x)^T) ---------
        with tc.tile_pool(name="pA", bufs=2) as pA, \
             tc.tile_pool(name="pAp", bufs=2, space="PSUM") as pAp:
            xlT = pA.tile([128, HALO], bf16, tag="xlT")
            for s in range(3):
                sl = slice(384 * s, 384 * (s + 1))
                nc.scalar.activation(xlT[:, sl], xT_sb[:, sl], AF.Prelu,
                                     bias=0.0, scale=1.0, alpha=a1_sb[:, 0:1])
                hp = pAp.tile([W, 384], f32, tag="hp")
                nc.tensor.matmul(hp[:], w_in_b[:], xlT[:, sl],
                                 start=True, stop=True)
                nc.scalar.activation(hT[:, sl], hp[:], AF.Prelu,
                                     bias=0.0, scale=1.0, alpha=a1_sb[0:W, 0:1])

        # ---------------- Phase A2: G slabs = hT_slice^T @ Wkoc ------------
        with tc.tile_pool(name="pG", bufs=2, space="PSUM") as pG:
            for t in range(NT):
                gp = pG.tile([128, GW], f32, tag="gp")
                nc.tensor.matmul(gp[:, 0:512], hT[:, TS * t:TS * t + 128],
                                 wkoc_b[:, 0:512], start=True, stop=True,
                                 skip_group_check=True)
                nc.tensor.matmul(gp[:, 512:GW], hT[:, TS * t:TS * t + 128],
                                 wkoc_b[:, 512:GW], start=True, stop=True,
                                 skip_group_check=True)
                nc.scalar.copy(G[:, GW * t:GW * (t + 1)], gp[:])

        # ------ Phase A3: all tiles' center gathers nbc[m, k, t] ----------
        # nbc_all[m, 120*k + 12*t + d] = po[m + 8 - k, 12*t + d]
        nbc_all = pers.tile([128, K17 * NT * 12], f32, tag="nbc_all")
        with tc.tile_pool(name="pN", bufs=2, space="PSUM") as pN:
            for g in range(5):               # 4 k's per PSUM bank pass
                ks = range(4 * g, min(4 * g + 4, K17))
                np_ps = pN.tile([128, 480], f32, tag="np")
                for i, k in enumerate(ks):
                    nc.tensor.matmul(np_ps[:, 120 * i:120 * (i + 1)],
                                     shc_sb[:, 128 * k:128 * (k + 1)],
                                     po_all[:], start=True, stop=True,
                                     skip_group_check=True)
                nc.scalar.copy(nbc_all[:, 480 * g:480 * g + 120 * len(ks)],
                               np_ps[:, 0:120 * len(ks)])

        # ------ Phase A4: self-edge kern for all tiles --------------------
        # kself_all[m, 24t+c] = ncl(m,t) * lrelu(sum_a rn_a*w5r[a,c] + b5r[c])
        kself_all = pers.tile([128, NT * KC], bf16, tag="kself_all")
        with tc.tile_pool(name="pK", bufs=1) as pK:
            oriv = po_all[:].rearrange("p (t d) -> p t d", d=12)[:, :, 3:12]
            sqo = pK.tile([128, NT * 9], f32, tag="sqo")
            sqv = sqo[:].rearrange("p (t d) -> p t d", d=9)
            nc.vector.tensor_mul(sqv, oriv, oriv)
            rn_all = pK.tile([128, NT * 3], f32, tag="rn_all")
            nc.vector.tensor_reduce(
                rn_all[:], sqo[:].rearrange("p (t a b) -> p t a b", a=3, b=3),
                axis=AX.X, op=OP.add)
            rnv = rn_all[:].rearrange("p (t a) -> p t a", a=3)
            ps_all = pK.tile([128, NT * KC], f32, tag="ps_all")
            psv = ps_all[:].rearrange("p (t c) -> p t c", c=KC)
            tmp = pK.tile([128, NT * KC], f32, tag="tmpk")
            tmv = tmp[:].rearrange("p (t c) -> p t c", c=KC)
            nc.vector.tensor_mul(
                psv, rnv[:, :, 0:1].broadcast_to([128, NT, KC]),
                w5r_sb[:, 0:KC].unsqueeze(1).broadcast_to([128, NT, KC]))
            nc.vector.tensor_mul(
                tmv, rnv[:, :, 1:2].broadcast_to([128, NT, KC]),
                w5r_sb[:, KC:2 * KC].unsqueeze(1).broadcast_to([128, NT, KC]))
            nc.vector.tensor_add(ps_all[:], ps_all[:], tmp[:])
            nc.vector.tensor_mul(
                tmv, rnv[:, :, 2:3].broadcast_to([128, NT, KC]),
                w5r_sb[:, 2 * KC:3 * KC].unsqueeze(1).broadcast_to([128, NT, KC]))
            nc.vector.tensor_add(ps_all[:], ps_all[:], tmp[:])
            nc.vector.tensor_add(
                psv, psv, b5r_sb[:, :].unsqueeze(1).broadcast_to([128, NT, KC]))
            nc.vector.scalar_tensor_tensor(ps_all[:], ps_all[:], NEG_K,
                                           ps_all[:], OP.mult, OP.max)
            nc.vector.tensor_mul(
                kself_all[:].rearrange("p (t c) -> p t c", c=KC), psv,
                ncl_sb[:].unsqueeze(-1).broadcast_to([128, NT, KC]))

        # ---------------- Phase B: per output tile ------------------------
        wrk = ctx.enter_context(tc.tile_pool(name="wrk", bufs=3))
        tpool = ctx.enter_context(tc.tile_pool(name="tmp", bufs=6))
        ps = ctx.enter_context(tc.tile_pool(name="ps", bufs=1, space="PSUM"))
        psw = ctx.enter_context(tc.tile_pool(name="psw", bufs=2, space="PSUM"))
        ps2 = ctx.enter_context(tc.tile_pool(name="ps2", bufs=1, space="PSUM"))

        P = 128  # products/geometry live on all 128 partitions (m = n + k)
        for t in range(NT):
            nbv = nbc_all[:].rearrange("p (k td) -> p k td", td=NT * 12) \
                            [:, :, 12 * t:12 * (t + 1)]
            poN = po_all[:, 12 * t:12 * (t + 1)]     # neighbor = po[m]
            poN_pos = poN[:, 0:3]
            poN_ori = poN[:, 3:12]

            # ---- geometry -> delta_aug [P, (k,8)] for edge (m-k, k) ------
            da = wrk.tile([P, K17 * 8], f32, tag="da")
            dav = da[:].rearrange("p (k d) -> p k d", d=8)
            D = wrk.tile([P, K17 * 3], f32, tag="D")
            Dv = D[:].rearrange("p (k a) -> p k a", a=3)
            nc.vector.tensor_sub(Dv,
                                 poN_pos.unsqueeze(1).broadcast_to([P, K17, 3]),
                                 nbv[:, :, 0:3])
            sq = wrk.tile([P, K17 * 3], f32, tag="sq")
            nc.vector.tensor_mul(sq[:], D[:], D[:])
            d2 = wrk.tile([P, K17], f32, tag="d2")
            nc.vector.tensor_reduce(d2[:], sq[:].rearrange("p (k a) -> p k a", a=3),
                                    axis=AX.X, op=OP.add)
            nc.scalar.activation(dav[:, :, 6], d2[:], AF.Sqrt, bias=0.0,
                                 scale=1.0 / (R * R))
            dist = wrk.tile([P, K17], f32, tag="dist")
            nc.scalar.activation(dist[:], d2[:], AF.Sqrt, bias=0.0, scale=1.0)
            rec = wrk.tile([P, K17], f32, tag="rec")
            nc.vector.tensor_scalar_add(dist[:], dist[:], 1e-9)
            nc.vector.reciprocal(rec[:], dist[:])
            dirn = wrk.tile([P, K17 * 3], f32, tag="dirn")
            dirnv = dirn[:].rearrange("p (k a) -> p k a", a=3)
            nc.vector.tensor_mul(dirnv, Dv,
                                 rec[:].unsqueeze(-1).broadcast_to([P, K17, 3]))
            # local_a = sum_b Ri[a,b] * dirn[b]; Ri = center ori (from nbc)
            lm = wrk.tile([P, K17 * 9], f32, tag="lm")
            lmv = lm[:].rearrange("p (k a b) -> p k a b", a=3, b=3)
            nc.gpsimd.tensor_mul(
                lmv,
                nbv[:, :, 3:12].rearrange("p k (a b) -> p k a b", b=3),
                dirn[:].rearrange("p (k b) -> p k b", b=3).unsqueeze(2)
                       .broadcast_to([P, K17, 3, 3]))
            nc.vector.tensor_reduce(dav[:, :, 0:3], lmv, axis=AX.X, op=OP.add)
            # ofeat_a = sum_b Ri[a,b] * Rj[a,b]; Rj = neighbor ori (= po[m])
            ofm = wrk.tile([P, K17 * 9], f32, tag="ofm")
            ofmv = ofm[:].rearrange("p (k a b) -> p k a b", a=3, b=3)
            nc.gpsimd.tensor_mul(
                ofmv,
                nbv[:, :, 3:12].rearrange("p k (a b) -> p k a b", b=3),
                poN_ori.rearrange("p (a b) -> p a b", b=3).unsqueeze(1)
                       .broadcast_to([P, K17, 3, 3]))
            nc.vector.tensor_reduce(dav[:, :, 3:6], ofmv, axis=AX.X, op=OP.add)
            nc.vector.memset(dav[:, :, 7], 1.0)
            nc.gpsimd.tensor_mul(
                dav, dav,
                mask_sb[:, K17 * t:K17 * (t + 1)].unsqueeze(-1)
                      .broadcast_to([P, K17, 8]))

            # ---- kern2[m,(k,c)] = lrelu(delta @ WS) = kern(m-k, k, c) ----
            dT_ps = ps.tile([128, 256], f32, tag="dT")
            nc.tensor.matmul(dT_ps[:, 0:128], da[:, 0:128], id_sb[:, :],
                             is_transpose=True, start=True, stop=False,
                             skip_group_check=True)
            nc.tensor.matmul(dT_ps[0:8, 128:256], da[:, 128:136], id_sb[:, :],
                             is_transpose=True, start=False, stop=True,
                             skip_group_check=True)
            dT = wrk.tile([128, 256], bf16, tag="dT_sb")
            nc.scalar.copy(dT[:], dT_ps[:])
            pre_ps = ps.tile([P, K17 * KC], f32, tag="pre")
            nc.tensor.matmul(pre_ps[:], dT[:, 0:128], ws_a_b[:], start=True,
                             stop=False, skip_group_check=True)
            nc.tensor.matmul(pre_ps[:], dT[0:8, 128:256], ws_b_b[:], start=False,
                             stop=True, skip_group_check=True)
            kern = wrk.tile([P, K17 * KC], bf16, tag="kern")
            nc.scalar.activation(kern[:], pre_ps[:], AF.Prelu, bias=0.0,
                                 scale=1.0, alpha=a2_sb[:, 0:1])

            # ---- self-edge compensation (precomputed per core) -----------
            nc.gpsimd.tensor_add(kern[:, 8 * KC:9 * KC],
                                 kern[:, 8 * KC:9 * KC],
                                 kself_all[:, KC * t:KC * (t + 1)])

            # ---- tm_k[m] = G[m] * kern2[m, k]; PE shift-accumulate -------
            wide_ps = psw.tile([TS, GW], f32, tag="wide")

            def mult(eng, k, tag):
                tm = tpool.tile([P, GW], bf16, tag=tag)
                eng.tensor_mul(
                    tm[:].rearrange("p (o c) -> p o c", c=KC),
                    G[:, GW * t:GW * (t + 1)]
                        .rearrange("p (o c) -> p o c", c=KC),
                    kern[:, KC * k:KC * (k + 1)].unsqueeze(1)
                        .broadcast_to([P, W, KC]))
                return tm

            def acc(k, tm):
                nc.tensor.matmul(wide_ps[:, 0:512],
                                 shs_b[:, TS * k:TS * (k + 1)], tm[:, 0:512],
                                 start=(k == 0), stop=(k == K17 - 1),
                                 skip_group_check=True)
                nc.tensor.matmul(wide_ps[:, 512:GW],
                                 shs_b[:, TS * k:TS * (k + 1)], tm[:, 512:GW],
                                 start=(k == 0), stop=(k == K17 - 1),
                                 skip_group_check=True)

            probe = tpool.tile([P, GW], bf16, tag="probe")
            nc.vector.tensor_mul(
                probe[:].rearrange("p (c o) -> p c o", o=W),
                G[:, GW * t:GW * (t + 1)].rearrange("p (c o) -> p c o", o=W),
                kern[:, 0:KC].unsqueeze(-1).broadcast_to([P, KC, W]))
            for k in range(K17):
                acc(k, mult(nc.vector, k, "tm"))
            conv = wrk.tile([TS, W], f32, tag="conv")
            nc.vector.tensor_reduce(conv[:],
                                    wide_ps[:].rearrange("p (o c) -> p o c", c=KC),
                                    axis=AX.X, op=OP.add)

            # ---- out = lrelu(conv) @ W_out + x ---------------------------
            ct_ps = ps2.tile([W, TS], f32, tag="ct")
            nc.tensor.matmul(ct_ps[:], conv[:], id_sb[0:TS, 0:TS],
                             is_transpose=True, start=True, stop=True,
                             skip_group_check=True)
            convLT = wrk.tile([W, TS], bf16, tag="convLT")
            nc.scalar.activation(convLT[:], ct_ps[:], AF.Prelu, bias=0.0,
                                 scale=1.0, alpha=a1_sb[0:W, 0:1])
            out_ps = ps2.tile([TS, C], f32, tag="out")
            nc.tensor.matmul(out_ps[:], convLT[:], w_out_b[:],
                             start=True, stop=True, skip_group_check=True)
            out_sb = wrk.tile([TS, C], f32, tag="out_sb")
            nc.vector.tensor_add(out_sb[:], out_ps[:],
                                 xc_all[0:TS, C * t:C * t + C])
            cnt = min(TS, NPC - TS * t)
            nc.sync.dma_start(y[TS * t:TS * t + cnt, :], out_sb[0:cnt, :])

    nc.compile()
    return nc


def _expected_src_dst():
    i = np.arange(N)
    offs = np.arange(-WIN, WIN + 1)
    j = i[:, None] + offs[None, :]
    valid = ((j // L) == (i[:, None] // L)) & (j >= 0) & (j < N)
    j = np.where(valid, j, i[:, None])
    dst = np.repeat(i, offs.size).astype(np.int32)
    src = j.reshape(-1).astype(np.int32)
    return src, dst


def _host_inputs(x, pos, ori, W_in, Ws0, bs0, Wk, W_out):
    xf = np.ascontiguousarray(x.reshape(N, C), np.float32)
    pos = np.asarray(pos, np.float32)
    ori = np.asarray(ori, np.float32)

    WS = np.zeros((136, K17 * KC), np.float32)
    for k in range(K17):
        s = _sidx(k)
        WS[8 * k:8 * k + 7, KC * k:KC * (k + 1)] = Ws0[s]
        WS[8 * k + 7, KC * k:KC * (k + 1)] = bs0[s]
    # wkoc[w, o*KC + c] = Wk[c*W + w, o]
    wkoc = np.ascontiguousarray(
        np.transpose(np.asarray(Wk, np.float32).reshape(KC, W, W),
                     (1, 2, 0)).reshape(W, GW))
    # center gather: nbc[m, k] = po[m + 8 - k]
    shifts_c = np.zeros((128, K17 * 128), np.float32)
    for k in range(K17):
        for m in range(128):
            r = m + 8 - k
            if 0 <= r < 128:
                shifts_c[r, 128 * k + m] = 1.0
    # shift-accumulate: out[n] += tm_k[n + k]
    shifts_s = np.zeros((128, K17 * TS), np.float32)
    for k in range(K17):
        for n in range(TS):
            shifts_s[n + k, TS * k + n] = 1.0
    w5r = np.tile(Ws0[5][3:6].reshape(1, 3 * KC), (128, 1)).astype(np.float32)
    b5r = np.tile(bs0[5].reshape(1, KC), (128, 1)).astype(np.float32)
    common = dict(
        w_in=np.ascontiguousarray(W_in, np.float32),
        ws_a=np.ascontiguousarray(WS[0:128]),
        ws_b=np.ascontiguousarray(WS[128:136]),
        wkoc=wkoc,
        w_out=np.ascontiguousarray(W_out, np.float32),
        ident=np.eye(128, dtype=np.float32),
        shifts_c=shifts_c,
        shifts_s=shifts_s,
        w5r=w5r, b5r=b5r,
        alph1=np.full((128, 1), NEG_IN, np.float32),
        alph2=np.full((128, 1), NEG_K, np.float32),
    )

    offs = np.arange(-WIN, WIN + 1)
    in_maps = []
    for ci in range(NCORES):
        s0 = ci * NPC
        g = s0 - WIN + np.arange(HALO)
        ok = (g >= 0) & (g < N)
        gi = np.clip(g, 0, N - 1)
        x_pad = np.where(ok[:, None], xf[gi], 0.0).astype(np.float32)
        p_pad = np.where(ok[:, None], pos[gi], 0.0).astype(np.float32)
        o_pad = np.where(ok[:, None], ori[gi], 0.0).astype(np.float32)

        xT_slot = np.ascontiguousarray(x_pad.T)                # [128, HALO]

        jj, pp = np.meshgrid(np.arange(NT), np.arange(128), indexing="ij")
        rows = (TS * jj + pp)            # [NT, 128] all < HALO
        po_pad = np.concatenate([p_pad, o_pad], axis=1)  # [HALO, 12]
        po_slot = po_pad[rows].transpose(1, 0, 2).reshape(128, NT * 12)
        rc = WIN + TS * jj + pp
        okc = rc < HALO
        xc_slot = np.where(okc[:, :, None], x_pad[np.minimum(rc, HALO - 1)], 0.0)
        xc_slot = xc_slot.transpose(1, 0, 2).reshape(128, NT * C).astype(np.float32)

        # mask2[m, t, k]: edge (n = m-k, k) exists; ncl2[m, t]: #folded
        # self-loops of node m-8 (kern2/kself live at partition m = n + k)
        mask = np.zeros((128, NT, K17), np.float32)
        ncl = np.zeros((128, NT), np.float32)
        for t in range(NT):
            cnt = min(TS, NPC - TS * t)
            for m in range(128):
                for k in range(K17):
                    n = m - k
                    if 0 <= n < cnt:
                        off = (s0 + TS * t + n) % L
                        if 0 <= off + k - WIN < L:
                            mask[m, t, k] = 1.0
                nn = m - WIN
                if 0 <= nn < cnt:
                    off = (s0 + TS * t + nn) % L
                    v = ((off + offs) >= 0) & ((off + offs) < L)
                    ncl[m, t] = K17 - v.sum()
        in_maps.append(dict(
            xT_slot=xT_slot, xc_slot=xc_slot, po_slot=po_slot,
            maskd=mask.reshape(128, NT * K17), nclmp=ncl, **common))
    return in_maps


def kernel(x, pos, seq, ori, W_in, Ws0, bs0, Wk, W_out, src, dst):
    exp_src, exp_dst = _expected_src_dst()
    assert np.array_equal(np.asarray(src), exp_src), "unexpected src graph"
    assert np.array_equal(np.asarray(dst), exp_dst), "unexpected dst graph"

    from concourse.bass_utils import run_bass_kernel_spmd

    if "nc" not in _PROG:
        _PROG["nc"] = _build_program()
    nc = _PROG["nc"]

    in_maps = _host_inputs(np.asarray(x), np.asarray(pos), np.asarray(ori),
                           np.asarray(W_in), np.asarray(Ws0), np.asarray(bs0),
                           np.asarray(Wk), np.asarray(W_out))
    res = run_bass_kernel_spmd(nc, in_maps, list(range(NCORES)))
    out = np.concatenate([res.results[i]["y"] for i in range(NCORES)], axis=0)
    return out.reshape(B, L, C).astype(np.float32)


# revision 17
# speedup vs baseline: 1.0953x; 1.0953x over previous
"""Bass/Trainium2 kernel for nn_BasicBlock_73933567033945 (CDConv / gnn_message_passing).

Strategy: graph is a fixed +-8 sequence window inside each of 4 chains of
L=2048 nodes (verified against the src/dst inputs at runtime). Shard the
8192 nodes across 8 NeuronCores (1024 nodes each) with an 8-node halo.

Per core:
  Phase A: hT[w, m] = lrelu(W_in^T @ lrelu(x)^T) computed directly in
    transposed form from a host-transposed x slab (no PE transposes).
  Phase A2: G[m, o*24+c] = sum_w hT[w, m] * Wk[c*32+w, o] (PE matmuls,
    hT slices stationary). G folds the output projection Wk into the
    gathered features, so the per-edge bilinear becomes
      conv[n, o] = sum_{k,c} kern[n, k, c] * G[n+k, o*24+c]
    (G slab row m of tile t holds node 112t + m - 8).
  Phase B per 112-node tile, products anchored at the G partition m:
    geometry is computed *center-shifted* -- the center pos|ori are
    gathered with down-shift matmuls so partition m holds delta of edge
    (n=m-k, k); the block-diagonal WS matmul then yields
    kern2[m, (k,c)] = kern(m-k, k, c) with no extra shifting. The DVE
    runs 17 all-bf16 contiguous multiplies tm_k[m] = G[m] * kern2[m,k],
    and 17 shift matmuls accumulate tm_k into PSUM re-aligned to the
    output node (out[n] += tm_k[n+k]), fusing the k-sum into the PE.
    A single 24-wide grouped reduce finishes the c-contraction.
    No per-k transposes, no Wk matmuls, no DVE add chains.
"""
import numpy as np

B, L, C = 4, 2048, 128
N = B * L
W = 32
KC = 24
SEQ_L = 11
R = 12.0
WIN = 8
NEG_IN = 0.1
NEG_K = 0.2
NCORES = 8
NPC = N // NCORES          # 1024 nodes per core
TS = 112                   # output nodes per tile
NT = 10                    # tiles per core (9*112 + 16)
HALO = 1152                # padded halo rows per core (needs 1136)
K17 = 2 * WIN + 1          # 17 window offsets
S_HALF = SEQ_L // 2
GW = W * KC                # 768 = width of a G slab / tm row

_PROG = {}


def _sidx(k):
    return int(np.clip(k - WIN, -S_HALF, S_HALF)) + S_HALF


def _build_program():
    import concourse.tile as tile
    from concourse import mybir, bacc
    from concourse.bass_utils import run_bass_kernel_spmd  # noqa: F401
    from contextlib import ExitStack

    f32 = mybir.dt.float32
    bf16 = mybir.dt.bfloat16
    AF = mybir.ActivationFunctionType
    OP = mybir.AluOpType
    AX = mybir.AxisListType

    nc = bacc.Bacc("TRN2", target_bir_lowering=False, debug=False)

    def din(name, shape):
        return nc.dram_tensor(name, shape, f32, kind="ExternalInput").ap()

    xT_slot = din("xT_slot", [128, HALO])
    xc_slot = din("xc_slot", [128, NT * C])
    po_slot = din("po_slot", [128, NT * 12])
    w_in = din("w_in", [C, W])
    ws_a = din("ws_a", [128, K17 * KC])
    ws_b = din("ws_b", [8, K17 * KC])
    wkoc = din("wkoc", [W, GW])
    w_out = din("w_out", [W, C])
    ident = din("ident", [128, 128])
    shifts_c = din("shifts_c", [128, K17 * 128])
    shifts_s = din("shifts_s", [128, K17 * TS])
    w5r = din("w5r", [128, 3 * KC])
    b5r = din("b5r", [128, KC])
    maskd = din("maskd", [128, NT * K17])
    nclmp = din("nclmp", [128, NT])
    alph1 = din("alph1", [128, 1])
    alph2 = din("alph2", [128, 1])
    y = nc.dram_tensor("y", [NPC, C], f32, kind="ExternalOutput").ap()

    with tile.TileContext(nc) as tc, ExitStack() as ctx:
        pers = ctx.enter_context(tc.tile_pool(name="pers", bufs=1))

        def load(ap_in, shape, tag):
            t = pers.tile(shape, f32, tag=tag)
            nc.sync.dma_start(t[:], ap_in)
            return t

        xT_sb = load(xT_slot, [128, HALO], "xT")
        xc_all = load(xc_slot, [128, NT * C], "xc_all")
        po_all = load(po_slot, [128, NT * 12], "po_all")
        w_in_f = load(w_in, [C, W], "w_in")
        ws_a_f = load(ws_a, [128, K17 * KC], "ws_a")
        ws_b_f = load(ws_b, [8, K17 * KC], "ws_b")
        wkoc_f = load(wkoc, [W, GW], "wkoc")
        w_out_f = load(w_out, [W, C], "w_out")
        id_sb = load(ident, [128, 128], "ident")
        shc_sb = load(shifts_c, [128, K17 * 128], "shifts_c")
        shs_f = load(shifts_s, [128, K17 * TS], "shifts_s")
        w5r_sb = load(w5r, [128, 3 * KC], "w5r")
        b5r_sb = load(b5r, [128, KC], "b5r")
        mask_sb = load(maskd, [128, NT * K17], "mask")
        ncl_sb = load(nclmp, [128, NT], "nclmp")
        a1_sb = load(alph1, [128, 1], "a1")
        a2_sb = load(alph2, [128, 1], "a2")

        # bf16 casts of shared weights
        w_in_b = pers.tile([C, W], bf16, tag="w_in_b")
        nc.vector.tensor_copy(w_in_b[:], w_in_f[:])
        ws_a_b = pers.tile([128, K17 * KC], bf16, tag="ws_a_b")
        nc.vector.tensor_copy(ws_a_b[:], ws_a_f[:])
        ws_b_b = pers.tile([8, K17 * KC], bf16, tag="ws_b_b")
        nc.vector.tensor_copy(ws_b_b[:], ws_b_f[:])
        wkoc_b = pers.tile([W, GW], bf16, tag="wkoc_b")
        nc.vector.tensor_copy(wkoc_b[:], wkoc_f[:])
        w_out_b = pers.tile([W, C], bf16, tag="w_out_b")
        nc.vector.tensor_copy(w_out_b[:], w_out_f[:])
        shs_b = pers.tile([128, K17 * TS], bf16, tag="shs_b")
        nc.vector.tensor_copy(shs_b[:], shs_f[:])

        hT = pers.tile([W, HALO], bf16, tag="hT")
        G = pers.tile([128, NT * GW], bf16, tag="G")

        # ---------------- Phase A: hT = lrelu(W_in^T @ lrelu(x)^T) ---------
        with tc.tile_pool(name="pA", bufs=2) as pA, \
             tc.tile_pool(name="pAp", bufs=2, space="PSUM") as pAp:
            xlT = pA.tile([128, HALO], bf16, tag="xlT")
            for s in range(3):
                sl = slice(384 * s, 384 * (s + 1))
                nc.scalar.activation(xlT[:, sl], xT_sb[:, sl], AF.Prelu,
                                     bias=0.0, scale=1.0, alpha=a1_sb[:, 0:1])
                hp = pAp.tile([W, 384], f32, tag="hp")
                nc.tensor.matmul(hp[:], w_in_b[:], xlT[:, sl],
                                 start=True, stop=True)
                nc.scalar.activation(hT[:, sl], hp[:], AF.Prelu,
                                     bias=0.0, scale=1.0, alpha=a1_sb[0:W, 0:1])

        # ---------------- Phase A2: G slabs = hT_slice^T @ Wkoc ------------
        with tc.tile_pool(name="pG", bufs=2, space="PSUM") as pG:
            for t in range(NT):
                gp = pG.tile([128, GW], f32, tag="gp")
                nc.tensor.matmul(gp[:, 0:512], hT[:, TS * t:TS * t + 128],
                                 wkoc_b[:, 0:512], start=True, stop=True,
                                 skip_group_check=True)
                nc.tensor.matmul(gp[:, 512:GW], hT[:, TS * t:TS * t + 128],
                                 wkoc_b[:, 512:GW], start=True, stop=True,
                                 skip_group_check=True)
                nc.scalar.copy(G[:, GW * t:GW * (t + 1)], gp[:])

        # ------ Phase A3: all tiles' center gathers nbc[m, k, t] ----------
        # nbc_all[m, 120*k + 12*t + d] = po[m + 8 - k, 12*t + d]
        nbc_all = pers.tile([128, K17 * NT * 12], f32, tag="nbc_all")
        with tc.tile_pool(name="pN", bufs=2, space="PSUM") as pN:
            for g in range(5):               # 4 k's per PSUM bank pass
                ks = range(4 * g, min(4 * g + 4, K17))
                np_ps = pN.tile([128, 480], f32, tag="np")
                for i, k in enumerate(ks):
                    nc.tensor.matmul(np_ps[:, 120 * i:120 * (i + 1)],
                                     shc_sb[:, 128 * k:128 * (k + 1)],
                                     po_all[:], start=True, stop=True,
                                     skip_group_check=True)
                nc.scalar.copy(nbc_all[:, 480 * g:480 * g + 120 * len(ks)],
                               np_ps[:, 0:120 * len(ks)])

        # ------ Phase A4: self-edge kern for all tiles --------------------
        # kself_all[m, 24t+c] = ncl(m,t) * lrelu(sum_a rn_a*w5r[a,c] + b5r[c])
        kself_all = pers.tile([128, NT * KC], bf16, tag="kself_all")
        with tc.tile_pool(name="pK", bufs=1) as pK:
            oriv = po_all[:].rearrange("p (t d) -> p t d", d=12)[:, :, 3:12]
            sqo = pK.tile([128, NT * 9], f32, tag="sqo")
            sqv = sqo[:].rearrange("p (t d) -> p t d", d=9)
            nc.vector.tensor_mul(sqv, oriv, oriv)
            rn_all = pK.tile([128, NT * 3], f32, tag="rn_all")
            nc.vector.tensor_reduce(
                rn_all[:], sqo[:].rearrange("p (t a b) -> p t a b", a=3, b=3),
                axis=AX.X, op=OP.add)
            rnv = rn_all[:].rearrange("p (t a) -> p t a", a=3)
            ps_all = pK.tile([128, NT * KC], f32, tag="ps_all")
            psv = ps_all[:].rearrange("p (t c) -> p t c", c=KC)
            tmp = pK.tile([128, NT * KC], f32, tag="tmpk")
            tmv = tmp[:].rearrange("p (t c) -> p t c", c=KC)
            nc.vector.tensor_mul(
                psv, rnv[:, :, 0:1].broadcast_to([128, NT, KC]),
                w5r_sb[:, 0:KC].unsqueeze(1).broadcast_to([128, NT, KC]))
            nc.vector.tensor_mul(
                tmv, rnv[:, :, 1:2].broadcast_to([128, NT, KC]),
                w5r_sb[:, KC:2 * KC].unsqueeze(1).broadcast_to([128, NT, KC]))
            nc.vector.tensor_add(ps_all[:], ps_all[:], tmp[:])
            nc.vector.tensor_mul(
                tmv, rnv[:, :, 2:3].broadcast_to([128, NT, KC]),
                w5r_sb[:, 2 * KC:3 * KC].unsqueeze(1).broadcast_to([128, NT, KC]))
            nc.vector.tensor_add(ps_all[:], ps_all[:], tmp[:])
            nc.vector.tensor_add(
                psv, psv, b5r_sb[:, :].unsqueeze(1).broadcast_to([128, NT, KC]))
            nc.vector.scalar_tensor_tensor(ps_all[:], ps_all[:], NEG_K,
                                           ps_all[:], OP.mult, OP.max)
            nc.vector.tensor_mul(
                kself_all[:].rearrange("p (t c) -> p t c", c=KC), psv,
                ncl_sb[:].unsqueeze(-1).broadcast_to([128, NT, KC]))

        # ---------------- Phase B: per output tile ------------------------
        wrk = ctx.enter_context(tc.tile_pool(name="wrk", bufs=3))
        tpool = ctx.enter_context(tc.tile_pool(name="tmp", bufs=6))
        ps = ctx.enter_context(tc.tile_pool(name="ps", bufs=1, space="PSUM"))
        psw = ctx.enter_context(tc.tile_pool(name="psw", bufs=2, space="PSUM"))
        ps2 = ctx.enter_context(tc.tile_pool(name="ps2", bufs=1, space="PSUM"))

        P = 128  # products/geometry live on all 128 partitions (m = n + k)
        pending_tail = None
        for t in range(NT):
            nbv = nbc_all[:].rearrange("p (k td) -> p k td", td=NT * 12) \
                            [:, :, 12 * t:12 * (t + 1)]
            poN = po_all[:, 12 * t:12 * (t + 1)]     # neighbor = po[m]
            poN_pos = poN[:, 0:3]
            poN_ori = poN[:, 3:12]

            # ---- geometry -> delta_aug [P, (k,8)] for edge (m-k, k) ------
            da = wrk.tile([P, K17 * 8], f32, tag="da")
            dav = da[:].rearrange("p (k d) -> p k d", d=8)
            D = wrk.tile([P, K17 * 3], f32, tag="D")
            Dv = D[:].rearrange("p (k a) -> p k a", a=3)
            nc.vector.tensor_sub(Dv,
                                 poN_pos.unsqueeze(1).broadcast_to([P, K17, 3]),
                                 nbv[:, :, 0:3])
            sq = wrk.tile([P, K17 * 3], f32, tag="sq")
            nc.vector.tensor_mul(sq[:], D[:], D[:])
            d2 = wrk.tile([P, K17], f32, tag="d2")
            nc.vector.tensor_reduce(d2[:], sq[:].rearrange("p (k a) -> p k a", a=3),
                                    axis=AX.X, op=OP.add)
            nc.scalar.activation(dav[:, :, 6], d2[:], AF.Sqrt, bias=0.0,
                                 scale=1.0 / (R * R))
            dist = wrk.tile([P, K17], f32, tag="dist")
            nc.scalar.activation(dist[:], d2[:], AF.Sqrt, bias=0.0, scale=1.0)
            rec = wrk.tile([P, K17], f32, tag="rec")
            nc.vector.tensor_scalar_add(dist[:], dist[:], 1e-9)
            nc.vector.reciprocal(rec[:], dist[:])
            dirn = wrk.tile([P, K17 * 3], f32, tag="dirn")
            dirnv = dirn[:].rearrange("p (k a) -> p k a", a=3)
            nc.vector.tensor_mul(dirnv, Dv,
                                 rec[:].unsqueeze(-1).broadcast_to([P, K17, 3]))
            # local_a = sum_b Ri[a,b] * dirn[b]; Ri = center ori (from nbc)
            lm = wrk.tile([P, K17 * 9], f32, tag="lm")
            lmv = lm[:].rearrange("p (k a b) -> p k a b", a=3, b=3)
            nc.gpsimd.tensor_mul(
                lmv,
                nbv[:, :, 3:12].rearrange("p k (a b) -> p k a b", b=3),
                dirn[:].rearrange("p (k b) -> p k b", b=3).unsqueeze(2)
                       .broadcast_to([P, K17, 3, 3]))
            nc.vector.tensor_reduce(dav[:, :, 0:3], lmv, axis=AX.X, op=OP.add)
            # ofeat_a = sum_b Ri[a,b] * Rj[a,b]; Rj = neighbor ori (= po[m])
            ofm = wrk.tile([P, K17 * 9], f32, tag="ofm")
            ofmv = ofm[:].rearrange("p (k a b) -> p k a b", a=3, b=3)
            nc.gpsimd.tensor_mul(
                ofmv,
                nbv[:, :, 3:12].rearrange("p k (a b) -> p k a b", b=3),
                poN_ori.rearrange("p (a b) -> p a b", b=3).unsqueeze(1)
                       .broadcast_to([P, K17, 3, 3]))
            nc.vector.tensor_reduce(dav[:, :, 3:6], ofmv, axis=AX.X, op=OP.add)
            nc.vector.memset(dav[:, :, 7], 1.0)
            nc.gpsimd.tensor_mul(
                dav, dav,
                mask_sb[:, K17 * t:K17 * (t + 1)].unsqueeze(-1)
                      .broadcast_to([P, K17, 8]))

            # ---- kern2[m,(k,c)] = lrelu(delta @ WS) = kern(m-k, k, c) ----
            dT_ps = ps.tile([128, 256], f32, tag="dT")
            nc.tensor.matmul(dT_ps[:, 0:128], da[:, 0:128], id_sb[:, :],
                             is_transpose=True, start=True, stop=False,
                             skip_group_check=True)
            nc.tensor.matmul(dT_ps[0:8, 128:256], da[:, 128:136], id_sb[:, :],
                             is_transpose=True, start=False, stop=True,
                             skip_group_check=True)
            dT = wrk.tile([128, 256], bf16, tag="dT_sb")
            nc.scalar.copy(dT[:], dT_ps[:])
            pre_ps = ps.tile([P, K17 * KC], f32, tag="pre")
            nc.tensor.matmul(pre_ps[:], dT[:, 0:128], ws_a_b[:], start=True,
                             stop=False, skip_group_check=True)
            nc.tensor.matmul(pre_ps[:], dT[0:8, 128:256], ws_b_b[:], start=False,
                             stop=True, skip_group_check=True)
            kern = wrk.tile([P, K17 * KC], bf16, tag="kern")
            nc.scalar.activation(kern[:], pre_ps[:], AF.Prelu, bias=0.0,
                                 scale=1.0, alpha=a2_sb[:, 0:1])

            # ---- self-edge compensation (precomputed per core) -----------
            nc.gpsimd.tensor_add(kern[:, 8 * KC:9 * KC],
                                 kern[:, 8 * KC:9 * KC],
                                 kself_all[:, KC * t:KC * (t + 1)])

            # ---- tm_k[m] = G[m] * kern2[m, k]; PE shift-accumulate -------
            if pending_tail is not None:
                pending_tail()
                pending_tail = None

            wide_ps = psw.tile([TS, GW], f32, tag="wide")

            def mult(eng, k, tag):
                tm = tpool.tile([P, GW], bf16, tag=tag)
                eng.tensor_mul(
                    tm[:].rearrange("p (o c) -> p o c", c=KC),
                    G[:, GW * t:GW * (t + 1)]
                        .rearrange("p (o c) -> p o c", c=KC),
                    kern[:, KC * k:KC * (k + 1)].unsqueeze(1)
                        .broadcast_to([P, W, KC]))
                return tm

            def acc(k, tm):
                nc.tensor.matmul(wide_ps[:, 0:512],
                                 shs_b[:, TS * k:TS * (k + 1)], tm[:, 0:512],
                                 start=(k == 0), stop=(k == K17 - 1),
                                 skip_group_check=True)
                nc.tensor.matmul(wide_ps[:, 512:GW],
                                 shs_b[:, TS * k:TS * (k + 1)], tm[:, 512:GW],
                                 start=(k == 0), stop=(k == K17 - 1),
                                 skip_group_check=True)

            for k in range(K17):
                acc(k, mult(nc.vector, k, "tm"))

            def make_tail(t, wide_ps):
                def tail():
                    # ---- conv c-reduce; out = lrelu(conv) @ W_out + x ----
                    conv = wrk.tile([TS, W], f32, tag="conv")
                    nc.vector.tensor_reduce(
                        conv[:], wide_ps[:].rearrange("p (o c) -> p o c", c=KC),
                        axis=AX.X, op=OP.add)
                    ct_ps = ps2.tile([W, TS], f32, tag="ct")
                    nc.tensor.matmul(ct_ps[:], conv[:], id_sb[0:TS, 0:TS],
                                     is_transpose=True, start=True, stop=True,
                                     skip_group_check=True)
                    convLT = wrk.tile([W, TS], bf16, tag="convLT")
                    nc.scalar.activation(convLT[:], ct_ps[:], AF.Prelu, bias=0.0,
                                         scale=1.0, alpha=a1_sb[0:W, 0:1])
                    out_ps = ps2.tile([TS, C], f32, tag="out")
                    nc.tensor.matmul(out_ps[:], convLT[:], w_out_b[:],
                                     start=True, stop=True, skip_group_check=True)
                    out_sb = wrk.tile([TS, C], f32, tag="out_sb")
                    nc.vector.tensor_add(out_sb[:], out_ps[:],
                                         xc_all[0:TS, C * t:C * t + C])
                    cnt = min(TS, NPC - TS * t)
                    nc.sync.dma_start(y[TS * t:TS * t + cnt, :], out_sb[0:cnt, :])
                return tail

            pending_tail = make_tail(t, wide_ps)
        pending_tail()

    nc.compile()
    return nc


def _expected_src_dst():
    i = np.arange(N)
    offs = np.arange(-WIN, WIN + 1)
    j = i[:, None] + offs[None, :]
    valid = ((j // L) == (i[:, None] // L)) & (j >= 0) & (j < N)
    j = np.where(valid, j, i[:, None])
    dst = np.repeat(i, offs.size).astype(np.int32)
    src = j.reshape(-1).astype(np.int32)
    return src, dst


def _host_inputs(x, pos, ori, W_in, Ws0, bs0, Wk, W_out):
    xf = np.ascontiguousarray(x.reshape(N, C), np.float32)
    pos = np.asarray(pos, np.float32)
    ori = np.asarray(ori, np.float32)

    WS = np.zeros((136, K17 * KC), np.float32)
    for k in range(K17):
        s = _sidx(k)
        WS[8 * k:8 * k + 7, KC * k:KC * (k + 1)] = Ws0[s]
        WS[8 * k + 7, KC * k:KC * (k + 1)] = bs0[s]
    # wkoc[w, o*KC + c] = Wk[c*W + w, o]
    wkoc = np.ascontiguousarray(
        np.transpose(np.asarray(Wk, np.float32).reshape(KC, W, W),
                     (1, 2, 0)).reshape(W, GW))
    # center gather: nbc[m, k] = po[m + 8 - k]
    shifts_c = np.zeros((128, K17 * 128), np.float32)
    for k in range(K17):
        for m in range(128):
            r = m + 8 - k
            if 0 <= r < 128:
                shifts_c[r, 128 * k + m] = 1.0
    # shift-accumulate: out[n] += tm_k[n + k]
    shifts_s = np.zeros((128, K17 * TS), np.float32)
    for k in range(K17):
        for n in range(TS):
            shifts_s[n + k, TS * k + n] = 1.0
    w5r = np.tile(Ws0[5][3:6].reshape(1, 3 * KC), (128, 1)).astype(np.float32)
    b5r = np.tile(bs0[5].reshape(1, KC), (128, 1)).astype(np.float32)
    common = dict(
        w_in=np.ascontiguousarray(W_in, np.float32),
        ws_a=np.ascontiguousarray(WS[0:128]),
        ws_b=np.ascontiguousarray(WS[128:136]),
        wkoc=wkoc,
        w_out=np.ascontiguousarray(W_out, np.float32),
        ident=np.eye(128, dtype=np.float32),
        shifts_c=shifts_c,
        shifts_s=shifts_s,
        w5r=w5r, b5r=b5r,
        alph1=np.full((128, 1), NEG_IN, np.float32),
        alph2=np.full((128, 1), NEG_K, np.float32),
    )

    offs = np.arange(-WIN, WIN + 1)
    in_maps = []
    for ci in range(NCORES):
        s0 = ci * NPC
        g = s0 - WIN + np.arange(HALO)
        ok = (g >= 0) & (g < N)
        gi = np.clip(g, 0, N - 1)
        x_pad = np.where(ok[:, None], xf[gi], 0.0).astype(np.float32)
        p_pad = np.where(ok[:, None], pos[gi], 0.0).astype(np.float32)
        o_pad = np.where(ok[:, None], ori[gi], 0.0).astype(np.float32)

        xT_slot = np.ascontiguousarray(x_pad.T)                # [128, HALO]

        jj, pp = np.meshgrid(np.arange(NT), np.arange(128), indexing="ij")
        rows = (TS * jj + pp)            # [NT, 128] all < HALO
        po_pad = np.concatenate([p_pad, o_pad], axis=1)  # [HALO, 12]
        po_slot = po_pad[rows].transpose(1, 0, 2).reshape(128, NT * 12)
        rc = WIN + TS * jj + pp
        okc = rc < HALO
        xc_slot = np.where(okc[:, :, None], x_pad[np.minimum(rc, HALO - 1)], 0.0)
        xc_slot = xc_slot.transpose(1, 0, 2).reshape(128, NT * C).astype(np.float32)

        # mask2[m, t, k]: edge (n = m-k, k) exists; ncl2[m, t]: #folded
        # self-loops of node m-8 (kern2/kself live at partition m = n + k)
        mask = np.zeros((128, NT, K17), np.float32)
        ncl = np.zeros((128, NT), np.float32)
        for t in range(NT):
            cnt = min(TS, NPC - TS * t)
            for m in range(128):
                for k in range(K17):
                    n = m - k
                    if 0 <= n < cnt:
                        off = (s0 + TS * t + n) % L
                        if 0 <= off + k - WIN < L:
                            mask[m, t, k] = 1.0
                nn = m - WIN
                if 0 <= nn < cnt:
                    off = (s0 + TS * t + nn) % L
                    v = ((off + offs) >= 0) & ((off + offs) < L)
                    ncl[m, t] = K17 - v.sum()
        in_maps.append(dict(
            xT_slot=xT_slot, xc_slot=xc_slot, po_slot=po_slot,
            maskd=mask.reshape(128, NT * K17), nclmp=ncl, **common))
    return in_maps


def kernel(x, pos, seq, ori, W_in, Ws0, bs0, Wk, W_out, src, dst):
    exp_src, exp_dst = _expected_src_dst()
    assert np.array_equal(np.asarray(src), exp_src), "unexpected src graph"
    assert np.array_equal(np.asarray(dst), exp_dst), "unexpected dst graph"

    from concourse.bass_utils import run_bass_kernel_spmd

    if "nc" not in _PROG:
        _PROG["nc"] = _build_program()
    nc = _PROG["nc"]

    in_maps = _host_inputs(np.asarray(x), np.asarray(pos), np.asarray(ori),
                           np.asarray(W_in), np.asarray(Ws0), np.asarray(bs0),
                           np.asarray(Wk), np.asarray(W_out))
    res = run_bass_kernel_spmd(nc, in_maps, list(range(NCORES)))
    out = np.concatenate([res.results[i]["y"] for i in range(NCORES)], axis=0)
    return out.reshape(B, L, C).astype(np.float32)
